# revision 39
# baseline (speedup 1.0000x reference)
"""AvgPool2d(64x64, stride 1, auto_pad-replicate) on TRN2, 8 NeuronCores.

Reference computes, per (n, c) plane X [256, 256]:
    inner = box_sum_64x64(X) / 4096            # [193, 193]
    out[io, jo] = inner[clamp(io-31, 0, 192), clamp(jo-31, 0, 192)]

Two banded matmul stages (inner = Bv^T @ X @ Bw with banded 0/1 matrices):
    stage A: matmul(lhsT=X_chunk [h,w], rhs=Bv [h,io]) -> Y^T [w, io]
    stage B: matmul(lhsT=Y^T_chunk [w,io], rhs=Bw [w,jo]) -> inner [io, jo]

Key optimizations over the naive version:
  * Band slices per 128-row contraction chunk have only 128 nonzero
    columns (not 193): each (k-chunk, m-chunk) streams 128+63+65=256
    columns instead of 2x193, cutting PE streaming cycles by 1/3.
  * Input is fp8 E3M4 (4 mantissa bits) quantized on the host with
    error diffusion along w: box sums of the quantization error
    telescope to boundary carries, so the end-to-end rel err stays
    ~3e-3 while input DMA bytes halve vs bf16.
  * PSUM evacuation fused over 2-plane groups (amortizes the fixed
    per-op overhead of DVE/ACT): VectorE drains Y, ScalarE drains out.
  * Output trimmed: the second io chunk only carries 65 valid
    partitions; it is DMA'd separately so no dead bytes ship to DRAM.

Sharding: pure data parallel, batch dim 16 -> 2 per core, 128 (n,c)
planes per core. No collectives.
"""

import ml_dtypes
import numpy as np

import concourse.bass as bass
import concourse.tile as tile
from concourse import mybir
from concourse.bass_utils import run_bass_kernel_spmd


N_CORES = 8
N, C, H, W = 16, 64, 256, 256
KPOOL = 64
PLANES_PER_CORE = (N // N_CORES) * C  # 128
OUT_I = H - KPOOL + 1  # 193 distinct output rows/cols
M2 = OUT_I - 128  # 65, second io chunk
PAD_LO = (H - OUT_I) // 2  # 31
PAD_HI = H - OUT_I - PAD_LO  # 32

X_DT = mybir.dt.float8e3
X_NP = ml_dtypes.float8_e3m4
MM_DT = mybir.dt.bfloat16
MM_NP = ml_dtypes.bfloat16
OUT_DT = mybir.dt.bfloat16

BATCH = 16  # planes per input DMA batch
GROUPS = PLANES_PER_CORE // 2  # 64 2-plane evacuation groups
PIPE = 3  # software-pipeline distance between stage A and B, in groups
FLUSH = 8  # groups per output DMA flush (16 planes)
N_WARM = 14  # PE warmup matmuls while the first input DMA lands


def _band(n: int, k: int, scale: float) -> np.ndarray:
    """B[i, o] = scale if o <= i < o + k else 0;  [n, n-k+1]."""
    m = n - k + 1
    b = np.zeros((n, m), dtype=np.float32)
    for o in range(m):
        b[o : o + k, o] = scale
    return b


def _diffuse_fp8(x: np.ndarray) -> np.ndarray:
    """Quantize to E3M4 with error feedback along the last axis.

    Box sums of the quantized tensor then differ from exact by only the
    boundary carries, ~5x less error than round-to-nearest.
    """
    out = np.empty(x.shape, dtype=X_NP)
    c = np.zeros(x.shape[:-1], dtype=np.float32)
    for j in range(x.shape[-1]):
        v = x[..., j] + c
        q = v.astype(X_NP)
        c = v - np.asarray(q, dtype=np.float32)
        out[..., j] = q
    return out


def _split_multiwaits(nc: bass.Bass) -> None:
    """Walrus codegen allows a single sync-wait slot per compute instruction.

    Tile's semaphore assignment can emit several; hoist the extras onto
    standalone NOPs (which lower to pure sequencer waits) in front of the
    instruction, on the same engine, preserving order and semantics.
    """
    f = nc.m.functions[0]
    for block in f.blocks:
        out = []
        for inst in block.instructions:
            si = inst.sync_info
            if si is not None and len(si.on_wait) > 1:
                waits = list(si.on_wait)
                for w in waits[:-1]:
                    nop = mybir.InstNoOp(name=f"WS-{nc.next_id()}", ins=[], outs=[])
                    nop.engine = inst.engine
                    nop.sync_info = mybir.SyncInfo(on_wait=[w], on_update=[])
                    out.append(nop)
                inst.sync_info = mybir.SyncInfo(
                    on_wait=[waits[-1]], on_update=list(si.on_update)
                )
            out.append(inst)
        block.instructions = out


def _build(split_waits: bool = True) -> bass.Bass:
    nc = bass.Bass()
    # partition-major layouts: x [r, plane, k, w]; h = k*128 + r
    x_ext = nc.declare_dram_parameter(
        "x", [128, PLANES_PER_CORE, 2, W], X_DT, isOutput=False
    )
    bv_ext = nc.declare_dram_parameter("bv", [H, OUT_I], X_DT, isOutput=False)
    bw_ext = nc.declare_dram_parameter("bw", [W, OUT_I], MM_DT, isOutput=False)
    # out: io chunk 0 (128 rows) and chunk 1 (65 rows) ship separately so no
    # dead bytes hit the saturated HBM write path
    outa_ext = nc.declare_dram_parameter(
        "outa", [128, PLANES_PER_CORE, OUT_I], OUT_DT, isOutput=True
    )
    outb_ext = nc.declare_dram_parameter(
        "outb", [M2, PLANES_PER_CORE, OUT_I], OUT_DT, isOutput=True
    )

    n_batches = PLANES_PER_CORE // BATCH

    with tile.TileContext(nc) as tc:
        with (
            tc.tile_pool(name="consts", bufs=1) as consts,
            tc.tile_pool(name="xin", bufs=4) as xpool,
            tc.tile_pool(name="ysb", bufs=PIPE + 4) as ypool_sb,
            tc.tile_pool(name="osb", bufs=4) as opool_sb,
            tc.tile_pool(name="yps", bufs=2, space="PSUM") as ypool_ps,
            tc.tile_pool(name="ops", bufs=2, space="PSUM") as opool_ps,
        ):
            x_tiles = [None] * n_batches
            y_ps = [None] * GROUPS
            y_sb = [None] * GROUPS
            o_sb = [None] * GROUPS

            def dma_in(b, splits=(0, BATCH)):
                if x_tiles[b] is None:
                    x_tiles[b] = xpool.tile([128, BATCH, 2, W], X_DT, name="x_sb")
                for lo, hi in zip(splits[:-1], splits[1:]):
                    nc.sync.dma_start(
                        out=x_tiles[b][:, lo:hi],
                        in_=x_ext[:, b * BATCH + lo : b * BATCH + hi, :, :],
                    )

            # eight planes of runway at the very head of the SP stream, ahead
            # of even the band-matrix consts, so the early groups never stutter
            dma_in(0, splits=(0, 8))
            warm_w = consts.tile([128, 128], X_DT)
            nc.gpsimd.memset(warm_w, 0.0)

            # Band matrices, rows split into 2 chunks of 128 partitions:
            # [r, k, o] with global row = 128*k + r.
            bv_sb = consts.tile([128, 2, OUT_I], X_DT)
            nc.sync.dma_start(
                out=bv_sb, in_=bv_ext[:, :].rearrange("(k r) o -> r k o", k=2)
            )
            bw_sb = consts.tile([128, 2, OUT_I], MM_DT)
            nc.sync.dma_start(
                out=bw_sb, in_=bw_ext[:, :].rearrange("(k r) o -> r k o", k=2)
            )
            dma_in(0, splits=(8, 16))

            # Keep the PE HAM-warm while the first input DMA is in flight:
            # dummy matmuls on a memset scratch tile (no DMA dependency, so
            # they start the moment the runtime preamble ends).
            warm_ps = opool_ps.tile(
                [128, OUT_I], mybir.dt.float32, name="warm_ps", tag="o_ps"
            )
            for _ in range(N_WARM):
                nc.tensor.matmul(
                    warm_ps[:, 0:128],
                    lhsT=warm_w,
                    rhs=warm_w,
                    start=True,
                    stop=True,
                )

            def banded_mms(out_col, lhsT_of_k, band_sb):
                """One output row-block: 3 matmuls streaming the nonzero
                128-col support of each contraction chunk of the band.
                out_col(c0, c1) -> PSUM slice for band columns [c0, c1)."""
                nc.tensor.matmul(
                    out_col(0, 128),
                    lhsT=lhsT_of_k(0),
                    rhs=band_sb[:, 0, 0:128],
                    start=True,
                    stop=False,
                )
                nc.tensor.matmul(
                    out_col(65, 128),
                    lhsT=lhsT_of_k(1),
                    rhs=band_sb[:, 1, 65:128],
                    start=False,
                    stop=True,
                )
                nc.tensor.matmul(
                    out_col(128, OUT_I),
                    lhsT=lhsT_of_k(1),
                    rhs=band_sb[:, 1, 128:OUT_I],
                    start=True,
                    stop=True,
                )

            def stage_a_group(g):
                b, p0 = divmod(2 * g, BATCH)
                if p0 == 0 and b > 0:
                    dma_in(b)
                x_sb = x_tiles[b]
                # 2-plane PSUM tile; each plane slot is one full 2 KiB bank
                y_ps[g] = ypool_ps.tile([128, 2, 512], mybir.dt.float32, name="y_ps")
                for s in range(2):
                    p = p0 + s
                    for m in range(2):  # w-chunk -> PSUM partitions
                        base = m * OUT_I
                        banded_mms(
                            lambda c0, c1, s=s, base=base: y_ps[g][
                                :, s, base + c0 : base + c1
                            ],
                            lambda k, p=p, m=m: x_sb[:, p, k, m * 128 : (m + 1) * 128],
                            bv_sb,
                        )

            def stage_b_group(g):
                # evacuate stage A PSUM (both slots in one strided DVE op)
                y_sb[g] = ypool_sb.tile([128, 2, 2 * OUT_I], MM_DT, name="y_sb")
                nc.vector.tensor_copy(y_sb[g], y_ps[g][:, :, 0 : 2 * OUT_I])
                y_ps[g] = None
                # [part, slot, c, 256]: slot = one 2 KiB bank, c = io chunk
                o_ps = opool_ps.tile([128, 2, 2, 256], mybir.dt.float32, name="o_ps")
                for s in range(2):
                    for c, mlen in ((0, 128), (1, M2)):  # io chunk -> partitions
                        banded_mms(
                            lambda c0, c1, s=s, c=c, mlen=mlen: o_ps[
                                0:mlen, s, c, c0:c1
                            ],
                            lambda k, s=s, c=c, mlen=mlen: y_sb[g][
                                :, s, k * OUT_I + c * 128 : k * OUT_I + c * 128 + mlen
                            ],
                            bw_sb,
                        )
                # o_sb keeps the two io chunks in separate contiguous regions
                # [part, c, plane, jo] so each flush reads long dense runs;
                # the strided transposed write is free on the ACT port.
                # partitions 65..127 of the c=1 region hold stale PSUM data;
                # they are evacuated (free) but never DMA'd.
                q, r = divmod(g, FLUSH)
                if r == 0:
                    o_sb[q] = opool_sb.tile(
                        [128, 2, 2 * FLUSH, OUT_I], OUT_DT, name="o_sb"
                    )
                nc.scalar.copy(
                    o_sb[q][:, :, 2 * r : 2 * r + 2, :].rearrange("p c s o -> p s c o"),
                    o_ps[:, :, :, 0:OUT_I],
                )
                # flush FLUSH groups from the otherwise-idle gpsimd (SWDGE)
                # queue: the sync ring stays dedicated to input so batches
                # never queue behind flushes, and the compute engines never
                # push descriptors
                # drain the final tile in slices so the end-of-kernel flush
                # has little left to move
                last_tile = q == GROUPS // FLUSH - 1
                flush_points = (
                    {FLUSH // 2 - 1: (0, FLUSH), FLUSH - 3: (FLUSH, FLUSH + 4),
                     FLUSH - 1: (FLUSH + 4, 2 * FLUSH)}
                    if last_tile
                    else {FLUSH - 1: (0, 2 * FLUSH)}
                )
                if r in flush_points:
                    lo, hi = flush_points[r]
                    p0 = 2 * (g - r)
                    nc.gpsimd.dma_start(
                        out=outa_ext[:, p0 + lo : p0 + hi, :],
                        in_=o_sb[q][:, 0, lo:hi, :],
                    )
                    nc.gpsimd.dma_start(
                        out=outb_ext[:, p0 + lo : p0 + hi, :],
                        in_=o_sb[q][0:M2, 1, lo:hi, :],
                    )
                    if r == FLUSH - 1:
                        o_sb[q] = None

            for g in range(GROUPS + PIPE):
                if g < GROUPS:
                    stage_a_group(g)
                if g >= PIPE:
                    stage_b_group(g - PIPE)

    if split_waits:
        _split_multiwaits(nc)
    return nc


_NC_CACHE = None


def _get_nc():
    global _NC_CACHE
    if _NC_CACHE is None:
        _NC_CACHE = _build()
    return _NC_CACHE


def _run(x: np.ndarray, trace: bool = False):
    x = np.asarray(x, dtype=np.float32)
    assert x.shape == (N, C, H, W), x.shape
    xq = _diffuse_fp8(x)
    # partition-major repack: [core, plane, (k r), w] -> [core, r, plane, k, w]
    xs = xq.reshape(N_CORES, PLANES_PER_CORE, 2, 128, W).transpose(0, 3, 1, 2, 4)
    xs = np.ascontiguousarray(xs)
    bv = _band(H, KPOOL, 1.0).astype(X_NP)
    bw = _band(W, KPOOL, 1.0 / (KPOOL * KPOOL)).astype(MM_NP)
    in_maps = [{"x": xs[i], "bv": bv, "bw": bw} for i in range(N_CORES)]
    # The device sporadically reports NRT_EXEC_UNIT_UNRECOVERABLE even for a
    # known-good NEFF; retry a couple of times before giving up.
    last_err = None
    for attempt in range(3):
        try:
            res = run_bass_kernel_spmd(
                nc=_get_nc(),
                in_maps=in_maps,
                core_ids=list(range(N_CORES)),
                trace=trace,
            )
            break
        except Exception as e:  # noqa: BLE001
            last_err = e
            import time

            time.sleep(2.0 * (attempt + 1))
    else:
        raise last_err
    # unpack: outa [128, plane, jo] rows io 0..127; outb [65, plane, jo]
    outs = []
    for i in range(N_CORES):
        oa = np.asarray(res.results[i]["outa"], dtype=np.float32)
        ob = np.asarray(res.results[i]["outb"], dtype=np.float32)
        inner = np.concatenate(
            [oa.transpose(1, 0, 2), ob.transpose(1, 0, 2)], axis=1
        )  # [plane, 193, 193]
        outs.append(inner)
    inner = np.stack(outs, axis=0)  # [cores, planes, 193, 193]
    full = np.pad(
        inner, ((0, 0), (0, 0), (PAD_LO, PAD_HI), (PAD_LO, PAD_HI)), mode="edge"
    )
    return full.reshape(N, C, H, W), res


def kernel(x: np.ndarray) -> np.ndarray:
    out, _ = _run(x, trace=False)
    return out


# revision 40
# speedup vs baseline: 1.1294x; 1.1294x over previous
"""AvgPool2d(64x64, stride 1, auto_pad-replicate) on TRN2, 8 NeuronCores.

Reference computes, per (n, c) plane X [256, 256]:
    inner = box_sum_64x64(X) / 4096            # [193, 193]
    out[io, jo] = inner[clamp(io-31, 0, 192), clamp(jo-31, 0, 192)]

Two banded matmul stages (inner = Bv^T @ X @ Bw with banded 0/1 matrices):
    stage A: matmul(lhsT=X_chunk [h,w], rhs=Bv [h,io]) -> Y^T [w, io]
    stage B: matmul(lhsT=Y^T_chunk [w,io], rhs=Bw [w,jo]) -> inner [io, jo]

Key optimizations over the naive version:
  * Band slices per 128-row contraction chunk have only 128 nonzero
    columns (not 193): each (k-chunk, m-chunk) streams 128+63+65=256
    columns instead of 2x193, cutting PE streaming cycles by 1/3.
  * Input is fp8 E3M4 (4 mantissa bits) quantized on the host with
    error diffusion along w: box sums of the quantization error
    telescope to boundary carries, so the end-to-end rel err stays
    ~3e-3 while input DMA bytes halve vs bf16.
  * PSUM evacuation fused over 2-plane groups (amortizes the fixed
    per-op overhead of DVE/ACT): VectorE drains Y, ScalarE drains out.
  * Output trimmed: the second io chunk only carries 65 valid
    partitions; it is DMA'd separately so no dead bytes ship to DRAM.

Sharding: pure data parallel, batch dim 16 -> 2 per core, 128 (n,c)
planes per core. No collectives.
"""

import ml_dtypes
import numpy as np

import concourse.bass as bass
import concourse.tile as tile
from concourse import mybir
from concourse.bass_utils import run_bass_kernel_spmd


N_CORES = 8
N, C, H, W = 16, 64, 256, 256
KPOOL = 64
PLANES_PER_CORE = (N // N_CORES) * C  # 128
OUT_I = H - KPOOL + 1  # 193 distinct output rows/cols
M2 = OUT_I - 128  # 65, second io chunk
PAD_LO = (H - OUT_I) // 2  # 31
PAD_HI = H - OUT_I - PAD_LO  # 32

X_DT = mybir.dt.float8e3
X_NP = ml_dtypes.float8_e3m4
MM_DT = mybir.dt.bfloat16
MM_NP = ml_dtypes.bfloat16
OUT_DT = mybir.dt.bfloat16

BATCH = 16  # planes per input DMA batch
GROUPS = PLANES_PER_CORE // 2  # 64 2-plane evacuation groups
PIPE = 4  # software-pipeline distance between stage A and B, in groups
FLUSH = 8  # groups per output DMA flush (16 planes)
N_WARM = 14  # PE warmup matmuls while the first input DMA lands


def _band(n: int, k: int, scale: float) -> np.ndarray:
    """B[i, o] = scale if o <= i < o + k else 0;  [n, n-k+1]."""
    m = n - k + 1
    b = np.zeros((n, m), dtype=np.float32)
    for o in range(m):
        b[o : o + k, o] = scale
    return b


def _diffuse_fp8(x: np.ndarray) -> np.ndarray:
    """Quantize to E3M4 with error feedback along the last axis.

    Box sums of the quantized tensor then differ from exact by only the
    boundary carries, ~5x less error than round-to-nearest.
    """
    out = np.empty(x.shape, dtype=X_NP)
    c = np.zeros(x.shape[:-1], dtype=np.float32)
    for j in range(x.shape[-1]):
        v = x[..., j] + c
        q = v.astype(X_NP)
        c = v - np.asarray(q, dtype=np.float32)
        out[..., j] = q
    return out


def _split_multiwaits(nc: bass.Bass) -> None:
    """Walrus codegen allows a single sync-wait slot per compute instruction.

    Tile's semaphore assignment can emit several; hoist the extras onto
    standalone NOPs (which lower to pure sequencer waits) in front of the
    instruction, on the same engine, preserving order and semantics.
    """
    f = nc.m.functions[0]
    for block in f.blocks:
        out = []
        for inst in block.instructions:
            si = inst.sync_info
            if si is not None and len(si.on_wait) > 1:
                waits = list(si.on_wait)
                for w in waits[:-1]:
                    nop = mybir.InstNoOp(name=f"WS-{nc.next_id()}", ins=[], outs=[])
                    nop.engine = inst.engine
                    nop.sync_info = mybir.SyncInfo(on_wait=[w], on_update=[])
                    out.append(nop)
                inst.sync_info = mybir.SyncInfo(
                    on_wait=[waits[-1]], on_update=list(si.on_update)
                )
            out.append(inst)
        block.instructions = out


def _build(split_waits: bool = True) -> bass.Bass:
    nc = bass.Bass()
    # partition-major layouts: x [r, plane, k, w]; h = k*128 + r
    x_ext = nc.declare_dram_parameter(
        "x", [128, PLANES_PER_CORE, 2, W], X_DT, isOutput=False
    )
    bv_ext = nc.declare_dram_parameter("bv", [H, OUT_I], X_DT, isOutput=False)
    bw_ext = nc.declare_dram_parameter("bw", [W, OUT_I], MM_DT, isOutput=False)
    # out: io chunk 0 (128 rows) and chunk 1 (65 rows) ship separately so no
    # dead bytes hit the saturated HBM write path
    outa_ext = nc.declare_dram_parameter(
        "outa", [128, PLANES_PER_CORE, OUT_I], OUT_DT, isOutput=True
    )
    outb_ext = nc.declare_dram_parameter(
        "outb", [M2, PLANES_PER_CORE, OUT_I], OUT_DT, isOutput=True
    )

    n_batches = PLANES_PER_CORE // BATCH

    with tile.TileContext(nc) as tc:
        with (
            tc.tile_pool(name="consts", bufs=1) as consts,
            tc.tile_pool(name="xin", bufs=4) as xpool,
            tc.tile_pool(name="ysb", bufs=PIPE + 4) as ypool_sb,
            tc.tile_pool(name="osb", bufs=4) as opool_sb,
            tc.tile_pool(name="yps", bufs=2, space="PSUM") as ypool_ps,
            tc.tile_pool(name="ops", bufs=2, space="PSUM") as opool_ps,
        ):
            x_tiles = [None] * n_batches
            y_ps = [None] * GROUPS
            y_sb = [None] * GROUPS
            o_sb = [None] * GROUPS

            def dma_in(b, splits=(0, BATCH)):
                if x_tiles[b] is None:
                    x_tiles[b] = xpool.tile([128, BATCH, 2, W], X_DT, name="x_sb")
                for lo, hi in zip(splits[:-1], splits[1:]):
                    nc.sync.dma_start(
                        out=x_tiles[b][:, lo:hi],
                        in_=x_ext[:, b * BATCH + lo : b * BATCH + hi, :, :],
                    )

            # eight planes of runway at the very head of the SP stream, ahead
            # of even the band-matrix consts, so the early groups never stutter
            dma_in(0, splits=(0, 8))
            warm_w = consts.tile([128, 128], X_DT)
            nc.gpsimd.memset(warm_w, 0.0)

            # Band matrices, rows split into 2 chunks of 128 partitions:
            # [r, k, o] with global row = 128*k + r.
            bv_sb = consts.tile([128, 2, OUT_I], X_DT)
            nc.sync.dma_start(
                out=bv_sb, in_=bv_ext[:, :].rearrange("(k r) o -> r k o", k=2)
            )
            bw_sb = consts.tile([128, 2, OUT_I], MM_DT)
            nc.sync.dma_start(
                out=bw_sb, in_=bw_ext[:, :].rearrange("(k r) o -> r k o", k=2)
            )
            dma_in(0, splits=(8, 16))

            # Keep the PE HAM-warm while the first input DMA is in flight:
            # dummy matmuls on a memset scratch tile (no DMA dependency, so
            # they start the moment the runtime preamble ends).
            warm_ps = opool_ps.tile(
                [128, OUT_I], mybir.dt.float32, name="warm_ps", tag="o_ps"
            )
            for _ in range(N_WARM):
                nc.tensor.matmul(
                    warm_ps[:, 0:128],
                    lhsT=warm_w,
                    rhs=warm_w,
                    start=True,
                    stop=True,
                )

            def banded_mms(out_col, lhsT_of_k, band_sb):
                """One output row-block: 3 matmuls streaming the nonzero
                128-col support of each contraction chunk of the band.
                out_col(c0, c1) -> PSUM slice for band columns [c0, c1)."""
                nc.tensor.matmul(
                    out_col(0, 128),
                    lhsT=lhsT_of_k(0),
                    rhs=band_sb[:, 0, 0:128],
                    start=True,
                    stop=False,
                )
                nc.tensor.matmul(
                    out_col(65, 128),
                    lhsT=lhsT_of_k(1),
                    rhs=band_sb[:, 1, 65:128],
                    start=False,
                    stop=True,
                )
                nc.tensor.matmul(
                    out_col(128, OUT_I),
                    lhsT=lhsT_of_k(1),
                    rhs=band_sb[:, 1, 128:OUT_I],
                    start=True,
                    stop=True,
                )

            def stage_a_group(g):
                b, p0 = divmod(2 * g, BATCH)
                if p0 == 0 and b > 0:
                    dma_in(b)
                x_sb = x_tiles[b]
                # 2-plane PSUM tile; each plane slot is one full 2 KiB bank
                y_ps[g] = ypool_ps.tile([128, 2, 512], mybir.dt.float32, name="y_ps")
                for s in range(2):
                    p = p0 + s
                    for m in range(2):  # w-chunk -> PSUM partitions
                        base = m * OUT_I
                        banded_mms(
                            lambda c0, c1, s=s, base=base: y_ps[g][
                                :, s, base + c0 : base + c1
                            ],
                            lambda k, p=p, m=m: x_sb[:, p, k, m * 128 : (m + 1) * 128],
                            bv_sb,
                        )

            def stage_b_group(g):
                # evacuate stage A PSUM (both slots in one strided DVE op)
                y_sb[g] = ypool_sb.tile([128, 2, 2 * OUT_I], MM_DT, name="y_sb")
                nc.vector.tensor_copy(y_sb[g], y_ps[g][:, :, 0 : 2 * OUT_I])
                y_ps[g] = None
                # [part, slot, c, 256]: slot = one 2 KiB bank, c = io chunk
                o_ps = opool_ps.tile([128, 2, 2, 256], mybir.dt.float32, name="o_ps")
                for s in range(2):
                    for c, mlen in ((0, 128), (1, M2)):  # io chunk -> partitions
                        banded_mms(
                            lambda c0, c1, s=s, c=c, mlen=mlen: o_ps[
                                0:mlen, s, c, c0:c1
                            ],
                            lambda k, s=s, c=c, mlen=mlen: y_sb[g][
                                :, s, k * OUT_I + c * 128 : k * OUT_I + c * 128 + mlen
                            ],
                            bw_sb,
                        )
                # o_sb keeps the two io chunks in separate contiguous regions
                # [part, c, plane, jo] so each flush reads long dense runs;
                # the strided transposed write is free on the ACT port.
                # partitions 65..127 of the c=1 region hold stale PSUM data;
                # they are evacuated (free) but never DMA'd.
                q, r = divmod(g, FLUSH)
                if r == 0:
                    o_sb[q] = opool_sb.tile(
                        [128, 2, 2 * FLUSH, OUT_I], OUT_DT, name="o_sb"
                    )
                nc.scalar.copy(
                    o_sb[q][:, :, 2 * r : 2 * r + 2, :].rearrange("p c s o -> p s c o"),
                    o_ps[:, :, :, 0:OUT_I],
                )
                # flush FLUSH groups from the otherwise-idle gpsimd (SWDGE)
                # queue: the sync ring stays dedicated to input so batches
                # never queue behind flushes, and the compute engines never
                # push descriptors
                # drain the final tile in slices so the end-of-kernel flush
                # has little left to move
                last_tile = q == GROUPS // FLUSH - 1
                flush_points = (
                    {FLUSH // 2 - 1: (0, FLUSH), FLUSH - 3: (FLUSH, FLUSH + 4),
                     FLUSH - 1: (FLUSH + 4, 2 * FLUSH)}
                    if last_tile
                    else {FLUSH - 1: (0, 2 * FLUSH)}
                )
                if r in flush_points:
                    lo, hi = flush_points[r]
                    p0 = 2 * (g - r)
                    nc.gpsimd.dma_start(
                        out=outa_ext[:, p0 + lo : p0 + hi, :],
                        in_=o_sb[q][:, 0, lo:hi, :],
                    )
                    nc.gpsimd.dma_start(
                        out=outb_ext[:, p0 + lo : p0 + hi, :],
                        in_=o_sb[q][0:M2, 1, lo:hi, :],
                    )
                    if r == FLUSH - 1:
                        o_sb[q] = None

            for g in range(GROUPS + PIPE):
                if g < GROUPS:
                    stage_a_group(g)
                if g >= PIPE:
                    stage_b_group(g - PIPE)

    if split_waits:
        _split_multiwaits(nc)
    return nc


_NC_CACHE = None


def _get_nc():
    global _NC_CACHE
    if _NC_CACHE is None:
        _NC_CACHE = _build()
    return _NC_CACHE


def _run(x: np.ndarray, trace: bool = False):
    x = np.asarray(x, dtype=np.float32)
    assert x.shape == (N, C, H, W), x.shape
    xq = _diffuse_fp8(x)
    # partition-major repack: [core, plane, (k r), w] -> [core, r, plane, k, w]
    xs = xq.reshape(N_CORES, PLANES_PER_CORE, 2, 128, W).transpose(0, 3, 1, 2, 4)
    xs = np.ascontiguousarray(xs)
    bv = _band(H, KPOOL, 1.0).astype(X_NP)
    bw = _band(W, KPOOL, 1.0 / (KPOOL * KPOOL)).astype(MM_NP)
    in_maps = [{"x": xs[i], "bv": bv, "bw": bw} for i in range(N_CORES)]
    # The device sporadically reports NRT_EXEC_UNIT_UNRECOVERABLE even for a
    # known-good NEFF; retry a couple of times before giving up.
    last_err = None
    for attempt in range(3):
        try:
            res = run_bass_kernel_spmd(
                nc=_get_nc(),
                in_maps=in_maps,
                core_ids=list(range(N_CORES)),
                trace=trace,
            )
            break
        except Exception as e:  # noqa: BLE001
            last_err = e
            import time

            time.sleep(2.0 * (attempt + 1))
    else:
        raise last_err
    # unpack: outa [128, plane, jo] rows io 0..127; outb [65, plane, jo]
    outs = []
    for i in range(N_CORES):
        oa = np.asarray(res.results[i]["outa"], dtype=np.float32)
        ob = np.asarray(res.results[i]["outb"], dtype=np.float32)
        inner = np.concatenate(
            [oa.transpose(1, 0, 2), ob.transpose(1, 0, 2)], axis=1
        )  # [plane, 193, 193]
        outs.append(inner)
    inner = np.stack(outs, axis=0)  # [cores, planes, 193, 193]
    full = np.pad(
        inner, ((0, 0), (0, 0), (PAD_LO, PAD_HI), (PAD_LO, PAD_HI)), mode="edge"
    )
    return full.reshape(N, C, H, W), res


def kernel(x: np.ndarray) -> np.ndarray:
    out, _ = _run(x, trace=False)
    return out
